# revision 7
# baseline (speedup 1.0000x reference)
"""Masked-BCE mean loss kernel for Trainium2, data-parallel over 8 NeuronCores.

Math (targets t are exactly 0.0/1.0, C=2 label columns):
    bce(x, t) = softplus(x) - x*t = softplus(y),  y = (1-2t)*x = w*x
    row mask  = 1[t0 + t1 > 0]
    answer    = sum(mask * (bce0 + bce1)) / (B*C)

Pair identity halves the Ln work:
    softplus(y0) + softplus(y1) = ln((1+e^{y0})(1+e^{y1})) = ln(1 + r),
    r = E0 + E1 + E0*E1,  E = e^y
With the pair mask multiplied into r before the Ln (bias=1.0), masked pairs
give ln(1) = 0, so the Ln's accum_out column IS the masked partial sum -- no
separate reduction, no PE work at all.

Per-core plan (shard = 2^21 elements as [128 x 16384] bf16; chunk cols are
[Y0h | Y1h] blocks so every DVE op is unit-stride bf16 -> 2x mode):
    DVE : Y  = W * X          (tensor_tensor, 2x; exact: w is +-1)
          V  = min(W0, W1)    (+1 on all-zero-target rows, -1 otherwise)
          M' = -0.5*V + 0.5   (tensor_scalar 2-op, 4x; exact {0,1})
          P2 = (E0 + 1) * E1  (scalar_tensor_tensor, 1x)
          r  = P2 + E0        (= (1+E0)(1+E1) - 1)
    GPS : rm = r * M'         (gpsimd tensor_tensor -- offloads DVE)
    ACT : E  = exp(Y)         (full pass)
          L  = ln(rm + 1), accum_out -> scol   (HALF pass; Exp+Ln pinned to
          the single `natural_log_exp_and_others` set -> one ACT_TABLE_LOAD)
Host: answer = sum(scol) / (B*C) in f64.
"""

import sys

import numpy as np

for _p in ("/opt/trn_rl_repo",):
    if _p not in sys.path:
        sys.path.insert(0, _p)

import concourse.tile as tile  # noqa: E402
from concourse import bacc, mybir  # noqa: E402
from concourse.bass_utils import run_bass_kernel_spmd  # noqa: E402

N_CORES = 8
B = 8388608
C = 2
PAIRS = B // N_CORES  # 1048576 pairs per core
P = 128
COLS = 2 * PAIRS // P  # 16384 total columns per core ([Y0|Y1] chunk blocks)
HCOLS = COLS // 2  # 8192 pair-columns per core

# chunk schedule in PAIR columns (h); first chunk small to prime the pipeline
CHUNK_H = [1024, 2048, 2560, 2560]
assert sum(CHUNK_H) == HCOLS
N_CHUNKS = len(CHUNK_H)

RM_ON_GPSIMD = True

dt = mybir.dt
AF = mybir.ActivationFunctionType
ALU = mybir.AluOpType

_CACHE: dict[str, object] = {}


def _patch_act_tables():
    """Pin Exp and Ln to the single covering table set (one ACT_TABLE_LOAD)."""
    if _CACHE.get("act_patched"):
        return
    import concourse.hw_specs as hw_specs

    orig = hw_specs.get_activation_tables

    def patched(module_arch):
        tabs = orig(module_arch)
        out = {}
        for name, funcs in tabs.items():
            if name == "natural_log_exp_and_others":
                out[name] = set(funcs)
            else:
                out[name] = set(funcs) - {AF.Exp, AF.Ln}
        return out

    bacc.get_activation_tables = patched
    _CACHE["act_patched"] = True


def _build_nc():
    _patch_act_tables()
    nc = bacc.Bacc(
        "TRN2", target_bir_lowering=False, debug=False, num_devices=N_CORES
    )
    x_d = nc.dram_tensor("x", [P, COLS], dt.bfloat16, kind="ExternalInput").ap()
    w_d = nc.dram_tensor("w", [P, COLS], dt.bfloat16, kind="ExternalInput").ap()
    scol_d = nc.dram_tensor(
        "scol", [P, N_CHUNKS], dt.float32, kind="ExternalOutput"
    ).ap()

    with tile.TileContext(nc) as tc:
        with (
            tc.tile_pool(name="io", bufs=3) as io_pool,
            tc.tile_pool(name="work", bufs=2) as work_pool,
            tc.tile_pool(name="outp", bufs=1) as out_pool,
        ):
            # tiny dummy Exp up front hoists the ~2.7us ACT_TABLE_LOAD off
            # the critical path (overlaps the first DMAs)
            warm = out_pool.tile([P, 8], dt.float32)
            nc.gpsimd.memset(warm[:], 0.0)
            nc.scalar.activation(warm[:], warm[:], AF.Exp)

            scol = out_pool.tile([P, N_CHUNKS], dt.float32)
            col0 = 0
            for ci, h in enumerate(CHUNK_H):
                f = 2 * h
                X = io_pool.tile([P, f], dt.bfloat16, tag="X")
                nc.sync.dma_start(X[:], x_d[:, col0 : col0 + f])
                W = io_pool.tile([P, f], dt.bfloat16, tag="W")
                nc.sync.dma_start(W[:], w_d[:, col0 : col0 + f])
                col0 += f

                Y = work_pool.tile([P, f], dt.bfloat16, tag="Y")
                nc.vector.tensor_tensor(Y[:], W[:], X[:], ALU.mult)

                E = work_pool.tile([P, f], dt.bfloat16, tag="E")
                nc.scalar.activation(E[:], Y[:], AF.Exp)

                V = work_pool.tile([P, h], dt.bfloat16, tag="V")
                nc.vector.tensor_tensor(V[:], W[:, :h], W[:, h:f], ALU.min)
                Mp = work_pool.tile([P, h], dt.bfloat16, tag="Mp")
                nc.vector.tensor_scalar(
                    Mp[:], V[:], -0.5, 0.5, ALU.mult, ALU.add
                )

                P2 = work_pool.tile([P, h], dt.bfloat16, tag="P2")
                nc.vector.scalar_tensor_tensor(
                    P2[:], E[:, :h], 1.0, E[:, h:f], ALU.add, ALU.mult
                )
                r = work_pool.tile([P, h], dt.bfloat16, tag="r")
                nc.vector.tensor_tensor(r[:], P2[:], E[:, :h], ALU.add)

                rm = work_pool.tile([P, h], dt.bfloat16, tag="rm")
                if RM_ON_GPSIMD:
                    nc.gpsimd.tensor_tensor(rm[:], r[:], Mp[:], ALU.mult)
                else:
                    nc.vector.tensor_tensor(rm[:], r[:], Mp[:], ALU.mult)

                L = work_pool.tile([P, h], dt.bfloat16, tag="L")
                nc.scalar.activation(
                    L[:], rm[:], AF.Ln, bias=1.0,
                    accum_out=scol[:, ci : ci + 1],
                )

            nc.sync.dma_start(scol_d[:], scol[:])

    nc.compile()
    return nc


def _get_nc():
    if "nc" not in _CACHE:
        _CACHE["nc"] = _build_nc()
    return _CACHE["nc"]


def _reduce_outputs(scols: list[np.ndarray]) -> np.ndarray:
    total = 0.0
    for sc in scols:
        total += sc.astype(np.float64).sum()  # sum(mask * (sp0 + sp1))
    return np.asarray(total / (B * C), dtype=np.float32)


def make_in_maps(inputs: np.ndarray, targets: np.ndarray) -> list[dict]:
    import ml_dtypes

    # Layout per core: pairs deinterleaved into [col0 | col1] blocks per chunk
    # so all device DVE ops are unit-stride (2x mode).  w = 1 - 2t (+-1, exact
    # in bf16) multiplies straight into x on device.
    x = np.ascontiguousarray(inputs, dtype=np.float32).reshape(
        N_CORES, PAIRS, C
    )
    w = 1.0 - 2.0 * np.ascontiguousarray(targets, dtype=np.float32).reshape(
        N_CORES, PAIRS, C
    )
    # [cores, C, P, HCOLS]: plane 0 = col-0 elements, plane 1 = col-1
    xp = x.transpose(0, 2, 1).reshape(N_CORES, C, P, HCOLS)
    wp = w.transpose(0, 2, 1).reshape(N_CORES, C, P, HCOLS)

    xs = np.empty((N_CORES, P, COLS), dtype=ml_dtypes.bfloat16)
    ws = np.empty((N_CORES, P, COLS), dtype=ml_dtypes.bfloat16)
    col0 = 0
    off = 0
    for h in CHUNK_H:
        xs[:, :, col0 : col0 + h] = xp[:, 0, :, off : off + h]
        xs[:, :, col0 + h : col0 + 2 * h] = xp[:, 1, :, off : off + h]
        ws[:, :, col0 : col0 + h] = wp[:, 0, :, off : off + h]
        ws[:, :, col0 + h : col0 + 2 * h] = wp[:, 1, :, off : off + h]
        col0 += 2 * h
        off += h
    return [{"x": xs[c], "w": ws[c]} for c in range(N_CORES)]


def kernel(inputs: np.ndarray, targets: np.ndarray) -> np.ndarray:
    nc = _get_nc()
    in_maps = make_in_maps(inputs, targets)
    res = run_bass_kernel_spmd(nc, in_maps, list(range(N_CORES)))
    scols = [res.results[c]["scol"] for c in range(N_CORES)]
    return _reduce_outputs(scols)


# revision 10
# speedup vs baseline: 1.2560x; 1.2560x over previous
"""Masked-BCE mean loss kernel for Trainium2, data-parallel over 8 NeuronCores.

Math (targets t are exactly 0.0/1.0, C=2 label columns):
    bce(x, t) = softplus(x) - x*t = softplus(y),  y = (1-2t)*x = w*x
    row mask  = 1[t0 + t1 > 0]
    answer    = sum(mask * (bce0 + bce1)) / (B*C)

Pair identity halves the Ln work:
    softplus(y0) + softplus(y1) = ln((1+e^{y0})(1+e^{y1})) = ln(1 + r),
    r = E0 + E1 + E0*E1,  E = e^y
With the pair mask multiplied into r before the Ln (bias=1.0), masked pairs
give ln(1) = 0, so the Ln's accum_out column IS the masked partial sum -- no
separate reduction, no PE work at all.

Per-core plan (shard = 2^21 elements as [128 x 16384] bf16; chunk cols are
[Y0h | Y1h] blocks so every DVE op is unit-stride bf16 -> 2x mode):
    DVE : Y  = W * X          (tensor_tensor, 2x; exact: w is +-1)
          V  = min(W0, W1)    (+1 on all-zero-target rows, -1 otherwise)
          M' = -0.5*V + 0.5   (tensor_scalar 2-op, 4x; exact {0,1})
          P2 = (E0 + 1) * E1  (scalar_tensor_tensor, 1x)
          r  = P2 + E0        (= (1+E0)(1+E1) - 1)
          rm = r * M'
    ACT : E  = exp(Y)         (full pass)
          L  = ln(rm + 1), accum_out -> scol   (HALF pass; Exp+Ln pinned to
          the single `natural_log_exp_and_others` set -> one ACT_TABLE_LOAD)

Engines run their queues IN ORDER, so emission is software-pipelined one
chunk deep: stage B(i+1) = {Y,V,M'} is emitted before stage C(i) = {P2,r,rm}
on DVE, and exp(i+1) before ln(i) on ACT -- a stalled ln/rm never blocks the
next chunk's independent work.  All input DMAs are issued up front (every
chunk has its own SBUF tile; no buffer-recycle waits).

Host: answer = sum(scol) / (B*C) in f64.
"""

import sys

import numpy as np

for _p in ("/opt/trn_rl_repo",):
    if _p not in sys.path:
        sys.path.insert(0, _p)

import concourse.tile as tile  # noqa: E402
from concourse import bacc, mybir  # noqa: E402
from concourse.bass_utils import run_bass_kernel_spmd  # noqa: E402

N_CORES = 8
B = 8388608
C = 2
PAIRS = B // N_CORES  # 1048576 pairs per core
P = 128
COLS = 2 * PAIRS // P  # 16384 total columns per core ([Y0|Y1] chunk blocks)
HCOLS = COLS // 2  # 8192 pair-columns per core

# chunk schedule in PAIR columns (h); first chunk smaller to prime the pipe
CHUNK_H = [1024, 1536, 1888, 1888, 1856]
assert sum(CHUNK_H) == HCOLS
N_CHUNKS = len(CHUNK_H)

dt = mybir.dt
AF = mybir.ActivationFunctionType
ALU = mybir.AluOpType

_CACHE: dict[str, object] = {}


def _patch_act_tables():
    """Pin Exp and Ln to the single covering table set (one ACT_TABLE_LOAD)."""
    if _CACHE.get("act_patched"):
        return
    import concourse.hw_specs as hw_specs

    orig = hw_specs.get_activation_tables

    def patched(module_arch):
        tabs = orig(module_arch)
        out = {}
        for name, funcs in tabs.items():
            if name == "natural_log_exp_and_others":
                out[name] = set(funcs)
            else:
                out[name] = set(funcs) - {AF.Exp, AF.Ln}
        return out

    bacc.get_activation_tables = patched
    _CACHE["act_patched"] = True


def _build_nc():
    _patch_act_tables()
    nc = bacc.Bacc(
        "TRN2", target_bir_lowering=False, debug=False, num_devices=N_CORES
    )
    x_d = nc.dram_tensor("x", [P, COLS], dt.bfloat16, kind="ExternalInput").ap()
    w_d = nc.dram_tensor("w", [P, COLS], dt.bfloat16, kind="ExternalInput").ap()
    scol_d = nc.dram_tensor(
        "scol", [P, N_CHUNKS], dt.float32, kind="ExternalOutput"
    ).ap()

    with tile.TileContext(nc) as tc:
        with (
            tc.tile_pool(name="io", bufs=N_CHUNKS) as io_pool,
            tc.tile_pool(name="work", bufs=2) as work_pool,
            tc.tile_pool(name="outp", bufs=1) as out_pool,
        ):
            # tiny dummy Exp up front hoists the ~2.7us ACT_TABLE_LOAD off
            # the critical path (overlaps the first DMAs)
            warm = out_pool.tile([P, 8], dt.float32)
            nc.vector.memset(warm[:], 0.0)
            nc.scalar.activation(warm[:], warm[:], AF.Exp)

            scol = out_pool.tile([P, N_CHUNKS], dt.float32)

            # all input DMAs up front; each chunk owns its tiles
            X, W = [], []
            col0 = 0
            for h in CHUNK_H:
                f = 2 * h
                Xc = io_pool.tile([P, f], dt.bfloat16, tag=f"X")
                nc.sync.dma_start(Xc[:], x_d[:, col0 : col0 + f])
                Wc = io_pool.tile([P, f], dt.bfloat16, tag=f"W")
                nc.sync.dma_start(Wc[:], w_d[:, col0 : col0 + f])
                X.append(Xc)
                W.append(Wc)
                col0 += f

            E = [None] * N_CHUNKS
            Mp = [None] * N_CHUNKS

            def stage_B(ci):  # input-side DVE + exp
                h = CHUNK_H[ci]
                f = 2 * h
                Y = work_pool.tile([P, f], dt.bfloat16, tag="Y")
                nc.vector.tensor_tensor(Y[:], W[ci][:], X[ci][:], ALU.mult)
                E[ci] = work_pool.tile(
                    [P, f], dt.bfloat16, tag="E", name=f"E{ci}"
                )
                nc.scalar.activation(E[ci][:], Y[:], AF.Exp)
                V = work_pool.tile([P, h], dt.bfloat16, tag="V")
                nc.vector.tensor_tensor(
                    V[:], W[ci][:, :h], W[ci][:, h:f], ALU.min
                )
                Mp[ci] = work_pool.tile(
                    [P, h], dt.bfloat16, tag="Mp", name=f"Mp{ci}"
                )
                nc.vector.tensor_scalar(
                    Mp[ci][:], V[:], -0.5, 0.5, ALU.mult, ALU.add
                )

            def stage_C(ci):  # pair combine + masked ln-accum
                h = CHUNK_H[ci]
                f = 2 * h
                Ec = E[ci]
                P2 = work_pool.tile([P, h], dt.bfloat16, tag="P2")
                nc.vector.scalar_tensor_tensor(
                    P2[:], Ec[:, :h], 1.0, Ec[:, h:f], ALU.add, ALU.mult
                )
                r = work_pool.tile([P, h], dt.bfloat16, tag="r")
                nc.vector.tensor_tensor(r[:], P2[:], Ec[:, :h], ALU.add)
                rm = work_pool.tile([P, h], dt.bfloat16, tag="rm")
                nc.vector.tensor_tensor(rm[:], r[:], Mp[ci][:], ALU.mult)
                L = work_pool.tile([P, h], dt.bfloat16, tag="L")
                nc.scalar.activation(
                    L[:], rm[:], AF.Ln, bias=1.0,
                    accum_out=scol[:, ci : ci + 1],
                )

            # software-pipelined emission, one chunk deep
            stage_B(0)
            for ci in range(1, N_CHUNKS):
                stage_B(ci)
                stage_C(ci - 1)
            stage_C(N_CHUNKS - 1)

            nc.sync.dma_start(scol_d[:], scol[:])

    nc.compile()
    return nc


def _get_nc():
    if "nc" not in _CACHE:
        _CACHE["nc"] = _build_nc()
    return _CACHE["nc"]


def _reduce_outputs(scols: list[np.ndarray]) -> np.ndarray:
    total = 0.0
    for sc in scols:
        total += sc.astype(np.float64).sum()  # sum(mask * (sp0 + sp1))
    return np.asarray(total / (B * C), dtype=np.float32)


def make_in_maps(inputs: np.ndarray, targets: np.ndarray) -> list[dict]:
    import ml_dtypes

    # Layout per core: pairs deinterleaved into [col0 | col1] blocks per chunk
    # so all device DVE ops are unit-stride (2x mode).  w = 1 - 2t (+-1, exact
    # in bf16) multiplies straight into x on device.
    x = np.ascontiguousarray(inputs, dtype=np.float32).reshape(
        N_CORES, PAIRS, C
    )
    w = 1.0 - 2.0 * np.ascontiguousarray(targets, dtype=np.float32).reshape(
        N_CORES, PAIRS, C
    )
    # [cores, C, P, HCOLS]: plane 0 = col-0 elements, plane 1 = col-1
    xp = x.transpose(0, 2, 1).reshape(N_CORES, C, P, HCOLS)
    wp = w.transpose(0, 2, 1).reshape(N_CORES, C, P, HCOLS)

    xs = np.empty((N_CORES, P, COLS), dtype=ml_dtypes.bfloat16)
    ws = np.empty((N_CORES, P, COLS), dtype=ml_dtypes.bfloat16)
    col0 = 0
    off = 0
    for h in CHUNK_H:
        xs[:, :, col0 : col0 + h] = xp[:, 0, :, off : off + h]
        xs[:, :, col0 + h : col0 + 2 * h] = xp[:, 1, :, off : off + h]
        ws[:, :, col0 : col0 + h] = wp[:, 0, :, off : off + h]
        ws[:, :, col0 + h : col0 + 2 * h] = wp[:, 1, :, off : off + h]
        col0 += 2 * h
        off += h
    return [{"x": xs[c], "w": ws[c]} for c in range(N_CORES)]


def kernel(inputs: np.ndarray, targets: np.ndarray) -> np.ndarray:
    nc = _get_nc()
    in_maps = make_in_maps(inputs, targets)
    res = run_bass_kernel_spmd(nc, in_maps, list(range(N_CORES)))
    scols = [res.results[c]["scol"] for c in range(N_CORES)]
    return _reduce_outputs(scols)
